# revision 1
# baseline (speedup 1.0000x reference)
import sys
sys.path.insert(0, '/opt/trn_rl_repo')
import numpy as np
import ml_dtypes

import concourse.bass as bass
import concourse.bacc as bacc
import concourse.mybir as mybir
from concourse import tile
from concourse.bass_utils import run_bass_kernel_spmd

BF16 = ml_dtypes.bfloat16
N, C, D, H, W = 8, 32, 64, 64, 64
NB = 256
CD = CH = CW = 16
NCORES = 8
BPC = NB // NCORES  # boxes per core

# imgq element strides for layout [n, z, y, Q(4), x, c8]
S_C, S_X, S_Q, S_Y, S_Z, S_N = 1, 8, 512, 2048, 131072, 8388608

last_exec_ns = None


def _axis_tables(lo, hi, L):
    # follows reference._coords/_lerp_idx in float32
    i = np.arange(CD, dtype=np.float32)
    step = (hi - lo) * (L - 1) / (CD - 1)
    coord = lo * (L - 1) + i * step
    coord = np.clip(coord, 0.0, L - 1)
    i0 = np.floor(coord).astype(np.int64)
    frac = (coord - i0).astype(np.float32)
    # remap i0 == L-1 so that i1 = i0+1 always stays in range
    sel = i0 == L - 1
    i0[sel] = L - 2
    frac[sel] = 1.0
    return i0, frac


def _pair_weights(iabs, i0, frac):
    # weight of absolute index iabs for each of the 16 outputs
    # iabs: [...]; i0/frac: [16]
    a = (iabs[..., None] == i0) * (1.0 - frac)
    b = (iabs[..., None] == i0 + 1) * frac
    return (a + b).astype(np.float32)


def kernel(image, boxes, box_ind):
    global last_exec_ns
    image = np.asarray(image, dtype=np.float32)
    boxes = np.asarray(boxes, dtype=np.float32)
    box_ind = np.asarray(box_ind)

    # ---- host: image relayout [N,C,D,H,W] -> [n,z,y,Q,x,c8] bf16 ----
    imgq = image.reshape(N, 4, 8, D, H, W).transpose(0, 3, 4, 1, 5, 2)
    imgq = np.ascontiguousarray(imgq).astype(BF16).reshape(-1)

    # ---- per-box geometry ----
    geos = []
    for b in range(NB):
        x1, y1, z1, x2, y2, z2 = boxes[b]
        z0, fz = _axis_tables(z1, z2, D)
        y0, fy = _axis_tables(y1, y2, H)
        x0, fx = _axis_tables(x1, x2, W)
        n = int(box_ind[b])
        wneed = int(x0.max() + 2 - x0.min())
        wbar = min(64, ((wneed + 15) // 16) * 16)
        xs = min(int(x0.min()), W - wbar)
        ysneed = int(y0.max() + 2 - y0.min())
        ybar = 32 if ysneed <= 32 else 64
        zneed = int(z0.max() + 2 - z0.min())
        geos.append(dict(n=n, z0=z0, fz=fz, y0=y0, fy=fy, x0=x0, fx=fx,
                         wbar=wbar, xs=xs, ybar=ybar, zneed=zneed, box=b))

    # sort by size desc, deal to (slot, core)
    order = sorted(range(NB), key=lambda b: -(geos[b]['zneed'] * geos[b]['ybar'] * geos[b]['wbar']))
    slot_boxes = [[order[s * NCORES + c] for c in range(NCORES)] for s in range(BPC)]

    # slot-uniform geometry
    slots = []
    for s in range(BPC):
        bs = [geos[b] for b in slot_boxes[s]]
        ybar = max(g['ybar'] for g in bs)
        m = 128 // ybar
        J = max(-(-g['zneed'] // m) for g in bs)
        J = min(J, 64 // m)
        wbar = max(g['wbar'] for g in bs)
        percore = []
        for g in bs:
            zlo = min(int(g['z0'].min()), D - J * m)
            ylo = min(int(g['y0'].min()), H - ybar)
            wb = wbar
            xs = min(g['xs'], W - wb)
            percore.append(dict(g=g, zlo=zlo, ylo=ylo, xs=xs))
        slots.append(dict(ybar=ybar, m=m, J=J, wbar=wbar, percore=percore,
                          big=(128 * J * 4 * wbar * 8 * 2) > (3 << 20)))

    # ---- per-core weight tables ----
    p_arr = np.arange(128)
    bts = [[] for _ in range(NCORES)]
    wxs = [[] for _ in range(NCORES)]
    bt_offs, wx_offs = [], []
    ob, ow = 0, 0
    for s, sl in enumerate(slots):
        J, m, ybar, wbar = sl['J'], sl['m'], sl['ybar'], sl['wbar']
        bt_offs.append(ob); wx_offs.append(ow)
        ob += J * 256; ow += (wbar // 16) * 128
        for c in range(NCORES):
            pc = sl['percore'][c]
            g = pc['g']
            zr = p_arr // ybar
            yr = p_arr % ybar
            # B [128, J, 256]
            zabs = pc['zlo'] + np.arange(J)[:, None] * m + zr[None, :]  # [J,128]
            wz = _pair_weights(zabs, g['z0'], g['fz'])                  # [J,128,16]
            wyv = _pair_weights(pc['ylo'] + yr, g['y0'], g['fy'])       # [128,16]
            B = np.einsum('jpz,py->pjzy', wz, wyv).reshape(128, J * 256)
            bts[c].append(B.astype(BF16))
            # Wx [128, (wbar//16)*128]: blk h: [r*8+c8, c8p*16+xo]
            xabs = pc['xs'] + np.arange(wbar)                            # [wbar]
            wxv = _pair_weights(xabs, g['x0'], g['fx'])                  # [wbar,16]
            nh = wbar // 16
            blk = np.zeros((nh, 16, 8, 8, 16), dtype=np.float32)
            for c8 in range(8):
                blk[:, :, c8, c8, :] = wxv.reshape(nh, 16, 16)
            wxs[c].append(blk.reshape(nh, 128, 128).transpose(1, 0, 2).reshape(128, nh * 128).astype(BF16))
    bt_np = [np.concatenate(bts[c], axis=1) for c in range(NCORES)]
    wx_np = [np.concatenate(wxs[c], axis=1) for c in range(NCORES)]
    TOTB, TOTW = bt_np[0].shape[1], wx_np[0].shape[1]

    # ---- build device program ----
    nc = bacc.Bacc("TRN2", target_bir_lowering=False, debug=False)
    img_t = nc.dram_tensor("img", [imgq.size], mybir.dt.bfloat16, kind="ExternalInput")
    bt_t = nc.dram_tensor("bt", [128, TOTB], mybir.dt.bfloat16, kind="ExternalInput")
    wx_t = nc.dram_tensor("wx", [128, TOTW], mybir.dt.bfloat16, kind="ExternalInput")
    out_t = nc.dram_tensor("out", [BPC, 128, 1024], mybir.dt.float32, kind="ExternalOutput")

    def slab_dmas(sl, c, G, s, Qs):
        # DMAs for core c, slot s, Q list Qs into tile G [128, J, len(Qs), wbar*8]
        J, m, ybar, wbar = sl['J'], sl['m'], sl['ybar'], sl['wbar']
        pc = sl['percore'][c]
        g = pc['g']
        base = g['n'] * S_N + pc['zlo'] * S_Z + pc['ylo'] * S_Y + pc['xs'] * S_X
        for qi, Q in enumerate(Qs):
            for zr in range(m):
                src = bass.AP(img_t, base + zr * S_Z + Q * S_Q,
                              [[S_Y, ybar], [S_Z * m, J], [S_X, wbar], [1, 8]])
                dst = G[zr * ybar:(zr + 1) * ybar, :, qi, :].rearrange(
                    "p j (x c) -> p j x c", c=8)
                nc.sync.dma_start(out=dst, in_=src)

    with tile.TileContext(nc) as tc:
        with tc.tile_pool(name="gf", bufs=2) as gfp, \
             tc.tile_pool(name="gq", bufs=2) as gqp, \
             tc.tile_pool(name="wt", bufs=2) as wtp, \
             tc.tile_pool(name="x1", bufs=2) as x1p, \
             tc.tile_pool(name="oo", bufs=2) as oop, \
             tc.tile_pool(name="ps", bufs=4, space="PSUM") as psp:
            cid = nc.sync.partition_id()
            for s, sl in enumerate(slots):
                J, m, ybar, wbar = sl['J'], sl['m'], sl['ybar'], sl['wbar']
                nh = wbar // 16
                btile = wtp.tile([128, J * 256], mybir.dt.bfloat16, tag="bt")
                nc.sync.dma_start(out=btile[:], in_=bt_t[:, bt_offs[s]:bt_offs[s] + J * 256])
                wtile = wtp.tile([128, nh * 128], mybir.dt.bfloat16, tag="wx")
                nc.sync.dma_start(out=wtile[:], in_=wx_t[:, wx_offs[s]:wx_offs[s] + nh * 128])
                O = oop.tile([128, 1024], mybir.dt.float32)
                qgroups = [[0], [1], [2], [3]] if sl['big'] else [[0, 1, 2, 3]]
                for Qs in qgroups:
                    G = (gqp if sl['big'] else gfp).tile(
                        [128, J, len(Qs), wbar * 8], mybir.dt.bfloat16,
                        tag="gq" if sl['big'] else "gf")
                    for k in range(NCORES):
                        with tc.If(cid == k):
                            slab_dmas(sl, k, G, s, Qs)
                    for qi, Q in enumerate(Qs):
                        X1 = x1p.tile([128, nh, 256], mybir.dt.bfloat16)
                        for h in range(nh):
                            psA = psp.tile([128, 256], mybir.dt.float32)
                            for j in range(J):
                                nc.tensor.matmul(
                                    out=psA[:],
                                    lhsT=G[:, j, qi, 128 * h:128 * (h + 1)],
                                    rhs=btile[:, 256 * j:256 * (j + 1)],
                                    start=(j == 0), stop=(j == J - 1))
                            nc.vector.tensor_copy(X1[:, h, :], psA[:])
                        psB = psp.tile([128, 256], mybir.dt.float32)
                        for h in range(nh):
                            nc.tensor.matmul(
                                out=psB[:], lhsT=wtile[:, 128 * h:128 * (h + 1)],
                                rhs=X1[:, h, :], start=(h == 0), stop=(h == nh - 1))
                        nc.vector.tensor_copy(O[:, 256 * Q:256 * (Q + 1)], psB[:])
                nc.sync.dma_start(out=out_t[s], in_=O[:])
    nc.compile()

    in_maps = [{"img": imgq, "bt": bt_np[c], "wx": wx_np[c]} for c in range(NCORES)]
    res = run_bass_kernel_spmd(nc, in_maps, list(range(NCORES)), trace=False)

    try:
        import os, time as _time
        if int(os.environ.get("BENCH_RETIME", "1")):
            from concourse import bass2jax as b2j
            best = None
            for _trial in range(2):
                t0 = _time.monotonic()
                b2j.run_bass_via_pjrt(nc, in_maps, n_cores=NCORES)
                dt = _time.monotonic() - t0
                best = dt if best is None else min(best, dt)
            last_exec_ns = int(best * 1e9)
        else:
            last_exec_ns = None
    except Exception:
        last_exec_ns = None

    # ---- host: reassemble ----
    out = np.zeros((NB, C, CD, CH, CW), dtype=np.float32)
    for s in range(BPC):
        for c in range(NCORES):
            b = slot_boxes[s][c]
            o = res.results[c]["out"][s]  # [128, 1024]
            # p = c8*16+xo ; free = Q*256 + zo*16 + yo
            o = o.reshape(8, 16, 4, 16, 16)          # [c8, xo, Q, zo, yo]
            out[b] = o.transpose(2, 0, 3, 4, 1).reshape(C, CD, CH, CW)
    return out

